# revision 24
# baseline (speedup 1.0000x reference)
"""Trainium2 Bass kernel for AdaptiveLogSoftmaxWithLoss (moe_routing).

Work split across the 8 cores (all fp8 DoubleRow GEMMs at 2 MACs/cell/cycle):
  - tails tensor-sharded by class columns (t0: 2000/core, t1: 3840/core,
    zero-padded on core 7) over host-compacted member rows only (the
    reference masks non-member rows; ~620 rows for t0, ~1260 for t1).
  - head sharded over (sample-tile quads x column halves): core pair
    (2j, 2j+1) owns sample tiles 4j..4j+3; even core takes head cols
    0:2048, odd core 2048:4096.  Fewer, larger ACT chunks than pure
    column sharding (the ACT fixed cost + accumulator read is ~1us/chunk).
  - hidden projections (h0, h1) replicated per core over compacted rows,
    fused descale+fp8 cast in one DVE op.

Target logits come from per-tile diagonal GEMMs: lhsT = the same
inpT/hidden fp8 slices, rhs = host-gathered fp8 weight rows of each
sample's target (zeroed on non-owner cores); a DVE (iota==p)*psum pass
extracts the diagonal with a dve-accumulator read.

Per-row sum-exp: one ACT exp+accum per PSUM chunk (2 x [128,2048] PSUM
slots rotate; PE fills one while ACT/DVE drain the other).  The tail1
B-chunks [1792] instead use a Schraudolph exp on the DVE (int32(x*K1+B)
bitcast to f32, mean-unbiased C) to balance the scalar and vector
engines; the host corrects core 7's zero-padded columns by the exact
approx-exp(0) value.

Host combine: sum partials over cores, lse = log(sum), NLL as in the
reference.  The host only shards, pads, quantizes, gathers and combines.
"""

import numpy as np
import ml_dtypes

import concourse.bass as bass
import concourse.bacc as bacc
import concourse.mybir as mybir
import concourse.tile as tile
from concourse.bass_utils import run_bass_kernel_spmd

BF16 = ml_dtypes.bfloat16
FP8 = ml_dtypes.float8_e4m3
H_SCALE = 8.0     # h cast to fp8 at 8x
W_SCALE = 64.0    # tail w2 cast to fp8 at 64x
IN_SCALE = 16.0   # inp cast to fp8 at 16x
W1_SCALE = 64.0   # w1 / head_w cast to fp8 at 64x
HID_DESCALE = 1.0 / (IN_SCALE * W1_SCALE)
DESCALE = 1.0 / (H_SCALE * W_SCALE)
WF_SCALE = 4096.0  # fused tail0 weight (w2@w1) fp8 scale
DESCALE0 = 1.0 / (IN_SCALE * WF_SCALE)
NCORES = 8
N, D = 2048, 1024
H0, H1 = 512, 256
C0, C1 = 4000, 20000
HEAD = 4002
HEAD_PAD = 4096
T0 = 16000
T1 = 30257
T1_PAD = 30720
WH, W0, W1 = HEAD_PAD // 2, T0 // 8, T1_PAD // 8     # 2048, 2000, 3840
MT = N // 128                                        # 16 sample tiles
PAD_H = HEAD_PAD - HEAD   # 94 zero cols, odd cores' half
PAD_1 = T1_PAD - T1       # 463 zero cols, core 7 (all in the B-chunk)

# Schraudolph exp: exp(ps*DESCALE) ~= bitcast_f32(int32(ps*SCH_K1 + SCH_B))
SCH_C = 473120.0          # tuned for zero mean relative bias
SCH_K1 = float(np.float32((2 ** 23) / np.log(2) / 512.0))
SCH_B = float(np.float32(1065353216.0 - SCH_C))
SCH_E0 = float(np.int32(np.float32(SCH_B)).view(np.float32))  # approx exp(0)

TRACE = False
EM_HEADS_IN_T1 = True
LAST_RESULT = None

_CACHED_NC = {}


def _chunks(total, step, off=0):
    out = []
    co = 0
    while co < total:
        out.append((off + co, min(step, total - co)))
        co += step
    return out


def _build_nc(P0, P1):
    global EM_HEADS_IN_T1
    N0, N1 = P0 * 128, P1 * 128
    nc = bacc.Bacc(None)
    BF = mybir.dt.bfloat16
    F8 = mybir.dt.float8e4
    F32 = mybir.dt.float32
    I32 = mybir.dt.int32
    OP = mybir.AluOpType
    ACTF = mybir.ActivationFunctionType
    DR = mybir.MatmulPerfMode.DoubleRow
    KT = D // 128

    # inputs packed into per-phase blobs: one DMA descriptor each (the
    # Sync/Scalar queues recycle ~8 DMA semaphores; many small dma_starts
    # serialize on sem reuse and wreck the ramp)
    nb2a = KT * (N0 + W0)
    nb3a = KT * (N1 + H1)
    nb3b = (H1 // 128) * (W1 + N1) + KT * N0
    nb4 = KT * (512 + WH + 512)
    cst_d = nc.dram_tensor("cst", [128, 129], F32, kind="ExternalInput")
    b2a_d = nc.dram_tensor("b2a", [128, nb2a], F8, kind="ExternalInput")
    b3a_d = nc.dram_tensor("b3a", [128, nb3a], F8, kind="ExternalInput")
    b3b_d = nc.dram_tensor("b3b", [128, nb3b], F8, kind="ExternalInput")
    b4_d = nc.dram_tensor("b4", [128, nb4], F8, kind="ExternalInput")
    ra_d = nc.dram_tensor("ra", [128, 4 + P0 + 2 * P1], F32, kind="ExternalOutput")
    rv_d = nc.dram_tensor("rv", [128, 4 + P0 + 2 * P1], F32, kind="ExternalOutput")

    with tile.TileContext(nc) as tc:
        with (
            tc.tile_pool(name="const", bufs=1) as cp,
            tc.tile_pool(name="work", bufs=3) as wp,
            tc.tile_pool(name="psum", bufs=2, space="PSUM") as bigp,
        ):
            cst = cp.tile([128, 129], F32)
            b2a = cp.tile([128, nb2a], F8)
            b3a = cp.tile([128, nb3a], F8)
            b3b = cp.tile([128, nb3b], F8)
            b4 = cp.tile([128, nb4], F8)
            h1T8 = cp.tile([128, H1 // 128, N1], F8)
            ra = cp.tile([128, 4 + P0 + 2 * P1], F32)
            rv = cp.tile([128, 4 + P0 + 2 * P1], F32)

            def _cut(blob, off, n, k):
                ap = blob[:, off : off + n * k]
                return ap.rearrange("p (k n) -> p k n", k=k), off + n * k

            pidx = cst[:, 0:1]
            iota = cst[:, 1:129]
            o = 0
            inp0T, o = _cut(b2a, o, N0, KT)
            w0fT, o = _cut(b2a, o, W0, KT)
            o = 0
            inp1T, o = _cut(b3a, o, N1, KT)
            w1t1, o = _cut(b3a, o, H1, KT)
            o = 0
            w2t1, o = _cut(b3b, o, W1, H1 // 128)
            wgT1, o = _cut(b3b, o, N1, H1 // 128)
            wgT0f, o = _cut(b3b, o, N0, KT)
            o = 0
            inpH, o = _cut(b4, o, 512, KT)
            hwT, o = _cut(b4, o, WH, KT)
            wgH, o = _cut(b4, o, 512, KT)
            resha = ra[:, 0:4]
            res0a = ra[:, 4 : 4 + P0]
            res1a = ra[:, 4 + P0 : 4 + P0 + 2 * P1].rearrange(
                "p (m c) -> p m c", m=P1
            )
            reshv = rv[:, 0:4]
            res0v = rv[:, 4 : 4 + P0]
            res1v = rv[:, 4 + P0 : 4 + P0 + 2 * P1].rearrange(
                "p (m c) -> p m c", m=P1
            )

            # loads in first-use order; k-pair interleave for the hidden0
            # path so its kt-major GEMM can start on the first pair
            nc.sync.dma_start(cst[:], cst_d[:])
            nc.sync.dma_start(b2a[:], b2a_d[:])
            nc.sync.dma_start(b3a[:], b3a_d[:])
            nc.sync.dma_start(b3b[:], b3b_d[:])
            nc.sync.dma_start(b4[:], b4_d[:])

            # preload the exp table during the DMA ramp
            warm = wp.tile([128, 1], BF, tag="warm")
            nc.scalar.activation(warm[:], pidx, ACTF.Exp)

            # warm the PE HAM clock gate during the DMA ramp (fp32 matmuls
            # on the tiny iota tile); the dummy DVE read frees the slot
            psw = bigp.tile([128, 2048], F32, tag="big", name="ps")
            for _ in range(40):
                nc.tensor.matmul(psw[:, :128], iota, iota, start=True, stop=True)
            wsink = wp.tile([128, 1], F32, tag="wsink")
            nc.vector.tensor_scalar_mul(wsink[:], psw[:, 0:1], 0.0)

            def mm_block(ps, fchunks, nkt, lhsT_fn, rhs_fn):
                kts = list(range(0, nkt, 2))
                for co, cw in fchunks:
                    for ki, kt in enumerate(kts):
                        nc.tensor.matmul(
                            ps[:, co : co + cw],
                            lhsT_fn(kt),
                            rhs_fn(kt, co, cw),
                            start=(ki == 0),
                            stop=(ki == len(kts) - 1),
                            perf_mode=DR,
                        )

            def hid_job(inT, w1, hT8, mh, width):
                ps = bigp.tile([128, 2048], F32, tag="big", name="ps")
                mm_block(
                    ps, _chunks(width, 512), KT,
                    lambda kt: w1[:, kt : kt + 2, mh * 128 : (mh + 1) * 128],
                    lambda kt, co, cw: inT[:, kt : kt + 2, co : co + cw],
                )
                nc.vector.tensor_scalar_mul(
                    hT8[:, mh, :], ps[:, :width], HID_DESCALE * H_SCALE
                )

            def exp_job(ps, cw, scale, s_ap):
                sc_e = wp.tile([128, 2048], BF, tag="sc_e")
                nc.scalar.activation(
                    sc_e[:, :cw], ps[:, :cw], ACTF.Exp, scale=scale, accum_out=s_ap
                )

            def schraud_job(ps, cw, s_ap, folds=1):
                e32 = wp.tile([128, 2048], I32, tag="e32")
                nc.vector.tensor_scalar(
                    out=e32[:, :cw], in0=ps[:, :cw],
                    scalar1=SCH_K1, scalar2=SCH_B,
                    op0=OP.mult, op1=OP.add,
                )
                # fold the bitcast exps on gpsimd (1 or 2 folds; its per-op
                # launch overhead is ~2us), final accum on the DVE
                ef = e32[:].bitcast(F32)
                t9 = wp.tile([128, 960], F32, tag="t9")
                nc.gpsimd.tensor_tensor(
                    out=t9[:], in0=ef[:, 0:960], in1=ef[:, 960:1920], op=OP.add
                )
                fw = 960
                if folds == 2:
                    u4 = wp.tile([128, 480], F32, tag="u4")
                    nc.gpsimd.tensor_tensor(
                        out=u4[:], in0=t9[:, 0:480], in1=t9[:, 480:960], op=OP.add
                    )
                    t9, fw = u4, 480
                sc2 = wp.tile([128, 960], BF, tag="sc2")
                nc.vector.tensor_scalar(
                    out=sc2[:, :fw], in0=t9[:, :fw],
                    scalar1=1.0, scalar2=0.0, op0=OP.mult, op1=OP.add,
                    accum_out=s_ap,
                )

            def t0_job(m):
                ms = slice(m * 128, (m + 1) * 128)
                ps = bigp.tile([128, 2048], F32, tag="big", name="ps")
                mm_block(
                    ps, _chunks(W0, 512), KT,
                    lambda kt: inp0T[:, kt : kt + 2, ms],
                    lambda kt, co, cw: w0fT[:, kt : kt + 2, co : co + cw],
                )
                exp_job(ps, W0, DESCALE0, res0a[:, m : m + 1])

            def dg0_batch():
                ps = bigp.tile([128, 2048], F32, tag="big", name="ps")
                for m in range(P0):
                    ms = slice(m * 128, (m + 1) * 128)
                    for ki, kt in enumerate(range(0, KT, 2)):
                        nc.tensor.matmul(
                            ps[:, m * 128 : (m + 1) * 128],
                            inp0T[:, kt : kt + 2, ms],
                            wgT0f[:, kt : kt + 2, ms],
                            start=(ki == 0), stop=(kt + 2 >= KT),
                            perf_mode=DR,
                        )
                sd = exp_blk(ps[:, : P0 * 128], P0 * 128, DESCALE0)
                for m in range(P0):
                    extract(sd[:, m * 128 : (m + 1) * 128], res0v[:, m : m + 1])

            def exp_blk(ps_blk, cw, scale):
                # diag blocks: exp into SBUF (no accum) so the PSUM slot is
                # released by ACT alone; DVE extracts lag off-path.  The host
                # recovers the logit as sum of ln over cores (non-owner cores
                # contribute exp(0)=1).
                sc_d = wp.tile([128, max(P0, P1, 6) * 128], F32, tag="sc_d")
                nc.scalar.activation(sc_d[:, :cw], ps_blk, ACTF.Exp, scale=scale)
                return sc_d

            def extract(sb_blk, t_ap):
                sc_g = wp.tile([128, 128], BF, tag="sc_g")
                nc.vector.scalar_tensor_tensor(
                    out=sc_g[:],
                    in0=iota,
                    scalar=pidx,
                    in1=sb_blk,
                    op0=OP.is_equal,
                    op1=OP.mult,
                    accum_out=t_ap,
                )

            HW1 = W1 // 2  # 1920: A-slot logit cols; diag block at 1920:2048

            def t1A_job(m):
                ms = slice(m * 128, (m + 1) * 128)
                psA = bigp.tile([128, 2048], F32, tag="big", name="ps")
                mm_block(
                    psA, _chunks(HW1, 512), H1 // 128,
                    lambda kt: h1T8[:, kt : kt + 2, ms],
                    lambda kt, co, cw: w2t1[:, kt : kt + 2, co : co + cw],
                )
                exp_job(psA, HW1, DESCALE, res1a[:, m, 0:1])

            def t1B_job(m):
                ms = slice(m * 128, (m + 1) * 128)
                psB = bigp.tile([128, 2048], F32, tag="big", name="ps")
                mm_block(
                    psB, _chunks(HW1, 512), H1 // 128,
                    lambda kt: h1T8[:, kt : kt + 2, ms],
                    lambda kt, co, cw: w2t1[:, kt : kt + 2, HW1 + co : HW1 + co + cw],
                )
                if m < P1 - 1:
                    schraud_job(psB, HW1, res1v[:, m, 0:1])
                else:
                    exp_job(psB, HW1, DESCALE, res1a[:, m, 1:2])

            def dg1_batch():
                ps = bigp.tile([128, 2048], F32, tag="big", name="ps")
                for m in range(P1):
                    ms = slice(m * 128, (m + 1) * 128)
                    nc.tensor.matmul(
                        ps[:, m * 128 : (m + 1) * 128],
                        h1T8[:, 0:2, ms],
                        wgT1[:, 0:2, ms],
                        start=True, stop=True, perf_mode=DR,
                    )
                sd = exp_blk(ps[:, : P1 * 128], P1 * 128, DESCALE)
                for m in range(P1):
                    extract(sd[:, m * 128 : (m + 1) * 128], res1v[:, m, 1:2])

            def head_job(lt):
                ls = slice(lt * 128, (lt + 1) * 128)
                ps = bigp.tile([128, 2048], F32, tag="big", name="ps")
                mm_block(
                    ps, _chunks(WH, 512), KT,
                    lambda kt: inpH[:, kt : kt + 2, ls],
                    lambda kt, co, cw: hwT[:, kt : kt + 2, co : co + cw],
                )
                exp_job(ps, WH, HID_DESCALE, resha[:, lt : lt + 1])

            def dgh_batch():
                ps = bigp.tile([128, 2048], F32, tag="big", name="ps")
                for lt in range(4):
                    ls = slice(lt * 128, (lt + 1) * 128)
                    for ki, kt in enumerate(range(0, KT, 2)):
                        nc.tensor.matmul(
                            ps[:, lt * 128 : (lt + 1) * 128],
                            inpH[:, kt : kt + 2, ls],
                            wgH[:, kt : kt + 2, ls],
                            start=(ki == 0), stop=(kt + 2 >= KT),
                            perf_mode=DR,
                        )
                sd = exp_blk(ps[:, :512], 512, HID_DESCALE)
                for lt in range(4):
                    extract(sd[:, lt * 128 : (lt + 1) * 128], reshv[:, lt : lt + 1])

            # t0 first (its data lands first), hid1 after two t0 jobs,
            # then t1 iters; heads woven into late t1 iters (EM_HEADS_IN_T1)
            # or as a tail phase
            with nc.named_scope("main"):
                seq = [lambda m=m: t0_job(m) for m in range(min(2, P0))]
                seq += [
                    lambda: hid_job(inp1T, w1t1, h1T8, 0, N1),
                    lambda: hid_job(inp1T, w1t1, h1T8, 1, N1),
                ]
                seq += [lambda m=m: t0_job(m) for m in range(2, P0)]
                seq.append(dg0_batch)
                heads = [lambda lt=lt: head_job(lt) for lt in range(4)]
                for m in range(P1):
                    seq.append(lambda m=m: t1A_job(m))
                    seq.append(lambda m=m: t1B_job(m))
                    if m == 4:
                        seq.append(dg1_batch)
                    if EM_HEADS_IN_T1 and m >= P1 - 5 and heads:
                        seq.append(heads.pop(0))
                seq += heads
                seq.append(dgh_batch)
                for f in seq:
                    f()

            nc.sync.dma_start(ra_d[:], ra[:])
            nc.scalar.dma_start(rv_d[:], rv[:])

    nc.finalize()
    return nc


def _get_nc(P0, P1):
    key = (P0, P1, EM_HEADS_IN_T1)
    if key not in _CACHED_NC:
        _CACHED_NC[key] = _build_nc(P0, P1)
    return _CACHED_NC[key]


def _tiled(a2d):
    """[K, F] (K multiple of 128) -> contiguous [128, K//128, F]."""
    K, F = a2d.shape
    return np.ascontiguousarray(
        a2d.reshape(K // 128, 128, F).transpose(1, 0, 2)
    )


def _unpm(a):
    """[128, m] -> [m*128]."""
    return np.ascontiguousarray(a.T).reshape(-1)


def make_in_maps(inp, tgt, head_w, t0_w1, t0_w2, t1_w1, t1_w2):
    inp = np.asarray(inp, dtype=np.float32)
    tgt = np.asarray(tgt).astype(np.int64)

    in1 = (tgt >= C0) & (tgt < C1)
    in2 = tgt >= C1
    idx0 = np.where(in1)[0]
    idx1 = np.where(in2)[0]
    n0, n1 = len(idx0), len(idx1)
    P0 = max(1, -(-n0 // 128))
    P1 = max(1, -(-n1 // 128))
    idx0p = np.concatenate([idx0, np.zeros(P0 * 128 - n0, np.int64)])
    idx1p = np.concatenate([idx1, np.zeros(P1 * 128 - n1, np.int64)])

    inpT_s = (inp.T * IN_SCALE).astype(FP8)           # [D, N]
    inp0T = _tiled(np.ascontiguousarray(inpT_s[:, idx0p]))
    inp1T = _tiled(np.ascontiguousarray(inpT_s[:, idx1p]))
    w1t1 = _tiled((np.asarray(t1_w1, np.float32).T * W1_SCALE).astype(FP8))
    # fused tail0 projection: logits = inp @ (w2 @ w1).T directly (k=1024)
    w0f = (np.asarray(t0_w2, np.float32) @ np.asarray(t0_w1, np.float32))
    w0fT_full = (w0f.T * WF_SCALE).astype(FP8)        # [D, T0]

    hwT_full = np.zeros((D, HEAD_PAD), FP8)
    hwT_full[:, :HEAD] = (np.asarray(head_w, np.float32).T * W1_SCALE).astype(FP8)
    w2t1_full = np.zeros((H1, T1_PAD), FP8)
    w2t1_full[:, :T1] = (np.asarray(t1_w2, np.float32).T * W_SCALE).astype(FP8)

    gi = np.where(tgt < C0, tgt, np.where(tgt < C1, C0, C0 + 1))
    rel0 = tgt[idx0p] - C0
    rel1 = tgt[idx1p] - C1

    def _gathT(full, rel, own):
        # [K, osz] -> gathered [K, nrows], zeroed on non-owner cores
        g = np.ascontiguousarray(full[:, np.clip(rel, 0, full.shape[1] - 1)])
        g[:, ~own] = 0
        return _tiled(g)

    iota = np.broadcast_to(
        np.arange(128, dtype=np.float32)[None, :], (128, 128)
    ).copy()
    pidx = np.arange(128, dtype=np.float32)[:, None].copy()

    def _flat(*tiles):
        return np.ascontiguousarray(
            np.concatenate([t.reshape(128, -1) for t in tiles], axis=1)
        )

    cst = np.concatenate([pidx, iota], axis=1).astype(np.float32)
    b3a = _flat(inp1T, w1t1)
    in_maps = []
    for i in range(NCORES):
        j, h = i // 2, i % 2
        smp = slice(j * 512, (j + 1) * 512)
        gih = gi[smp]
        wgH_full = np.ascontiguousarray(hwT_full[:, gih])
        if h == 1:
            wgH_full = np.zeros_like(wgH_full)
        in_maps.append(
            {
                "cst": cst,
                "b2a": _flat(
                    inp0T, _tiled(w0fT_full[:, i * W0 : (i + 1) * W0])
                ),
                "b3a": b3a,
                "b3b": _flat(
                    _tiled(w2t1_full[:, i * W1 : (i + 1) * W1]),
                    _gathT(w2t1_full, rel1, (rel1 // W1) == i),
                    _gathT(w0fT_full, rel0, (rel0 // W0) == i),
                ),
                "b4": _flat(
                    _tiled(np.ascontiguousarray(inpT_s[:, smp])),
                    _tiled(hwT_full[:, h * WH : (h + 1) * WH]),
                    _tiled(wgH_full),
                ),
            }
        )
    return in_maps, tgt, (idx0, idx1, n0, n1, P0, P1)


def combine(results, tgt, meta):
    """per-core {'resh','res0','res1'} partials -> final [N] f32 NLL."""
    idx0, idx1, n0, n1, P0, P1 = meta
    Sh = np.zeros((128, MT), np.float64)
    Th = np.zeros((128, MT), np.float64)
    S0 = np.zeros((128, P0), np.float64)
    T0s = np.zeros((128, P0), np.float64)
    S1 = np.zeros((128, P1), np.float64)
    T1s = np.zeros((128, P1), np.float64)
    for i, r in enumerate(results):
        j = i // 2
        ra = np.asarray(r["ra"], np.float64)
        rv = np.asarray(r["rv"], np.float64)
        res1a = ra[:, 4 + P0 :].reshape(128, P1, 2)
        res1v = rv[:, 4 + P0 :].reshape(128, P1, 2)
        Sh[:, 4 * j : 4 * j + 4] += ra[:, 0:4]
        Th[:, 4 * j : 4 * j + 4] += np.log(rv[:, 0:4])
        S0 += ra[:, 4 : 4 + P0]
        T0s += np.log(rv[:, 4 : 4 + P0])
        S1 += res1a[:, :, 0] + res1a[:, :, 1] + res1v[:, :, 0]
        T1s += np.log(res1v[:, :, 1])

    # zero-padded cols: head pad on odd cores' halves (exp(0)=1 each);
    # tail1 pad all in core 7's Schraudolph B-chunk (approx exp(0)=SCH_E0)
    head_term = _unpm(Th) - np.log(_unpm(Sh) - PAD_H)
    lp0 = _unpm(T0s) - np.log(_unpm(S0))
    padc = np.where(np.arange(P1) < P1 - 1, PAD_1 * SCH_E0, float(PAD_1))
    lp1 = _unpm(T1s) - np.log(_unpm(S1 - padc[None, :]))

    out = head_term
    out[idx0] += lp0[:n0]
    out[idx1] += lp1[:n1]
    return (-out).astype(np.float32)


def kernel(inp, tgt, head_w, t0_w1, t0_w2, t1_w1, t1_w2):
    global LAST_RESULT
    in_maps, tgt64, meta = make_in_maps(
        inp, tgt, head_w, t0_w1, t0_w2, t1_w1, t1_w2
    )
    nc = _get_nc(meta[4], meta[5])
    out = run_bass_kernel_spmd(
        nc, in_maps, core_ids=list(range(NCORES)), trace=TRACE
    )
    LAST_RESULT = out
    return combine(out.results, tgt64, meta)


# revision 25
# speedup vs baseline: 1.2125x; 1.2125x over previous
"""Trainium2 Bass kernel for AdaptiveLogSoftmaxWithLoss (moe_routing).

Work split across the 8 cores (all fp8 DoubleRow GEMMs at 2 MACs/cell/cycle):
  - tails tensor-sharded by class columns (t0: 2000/core, t1: 3840/core,
    zero-padded on core 7) over host-compacted member rows only (the
    reference masks non-member rows; ~620 rows for t0, ~1260 for t1).
  - head sharded over (sample-tile quads x column halves): core pair
    (2j, 2j+1) owns sample tiles 4j..4j+3; even core takes head cols
    0:2048, odd core 2048:4096.  Fewer, larger ACT chunks than pure
    column sharding (the ACT fixed cost + accumulator read is ~1us/chunk).
  - hidden projections (h0, h1) replicated per core over compacted rows,
    fused descale+fp8 cast in one DVE op.

Target logits come from per-tile diagonal GEMMs: lhsT = the same
inpT/hidden fp8 slices, rhs = host-gathered fp8 weight rows of each
sample's target (zeroed on non-owner cores); a DVE (iota==p)*psum pass
extracts the diagonal with a dve-accumulator read.

Per-row sum-exp: one ACT exp+accum per PSUM chunk (2 x [128,2048] PSUM
slots rotate; PE fills one while ACT/DVE drain the other).  The tail1
B-chunks [1792] instead use a Schraudolph exp on the DVE (int32(x*K1+B)
bitcast to f32, mean-unbiased C) to balance the scalar and vector
engines; the host corrects core 7's zero-padded columns by the exact
approx-exp(0) value.

Host combine: sum partials over cores, lse = log(sum), NLL as in the
reference.  The host only shards, pads, quantizes, gathers and combines.
"""

import numpy as np
import ml_dtypes

import concourse.bass as bass
import concourse.bacc as bacc
import concourse.mybir as mybir
import concourse.tile as tile
from concourse.bass_utils import run_bass_kernel_spmd

BF16 = ml_dtypes.bfloat16
FP8 = ml_dtypes.float8_e4m3
H_SCALE = 8.0     # h cast to fp8 at 8x
W_SCALE = 64.0    # tail w2 cast to fp8 at 64x
IN_SCALE = 16.0   # inp cast to fp8 at 16x
W1_SCALE = 64.0   # w1 / head_w cast to fp8 at 64x
HID_DESCALE = 1.0 / (IN_SCALE * W1_SCALE)
DESCALE = 1.0 / (H_SCALE * W_SCALE)
WF_SCALE = 4096.0  # fused tail0 weight (w2@w1) fp8 scale
DESCALE0 = 1.0 / (IN_SCALE * WF_SCALE)
NCORES = 8
N, D = 2048, 1024
H0, H1 = 512, 256
C0, C1 = 4000, 20000
HEAD = 4002
HEAD_PAD = 4096
T0 = 16000
T1 = 30257
T1_PAD = 30720
WH, W0, W1 = HEAD_PAD // 2, T0 // 8, T1_PAD // 8     # 2048, 2000, 3840
MT = N // 128                                        # 16 sample tiles
PAD_H = HEAD_PAD - HEAD   # 94 zero cols, odd cores' half
PAD_1 = T1_PAD - T1       # 463 zero cols, core 7 (all in the B-chunk)

# Schraudolph exp: exp(ps*DESCALE) ~= bitcast_f32(int32(ps*SCH_K1 + SCH_B))
SCH_C = 473120.0          # tuned for zero mean relative bias
SCH_K1 = float(np.float32((2 ** 23) / np.log(2) / 512.0))
SCH_B = float(np.float32(1065353216.0 - SCH_C))
SCH_E0 = float(np.int32(np.float32(SCH_B)).view(np.float32))  # approx exp(0)

TRACE = False
EM_HEADS_IN_T1 = False
LAST_RESULT = None

_CACHED_NC = {}


def _chunks(total, step, off=0):
    out = []
    co = 0
    while co < total:
        out.append((off + co, min(step, total - co)))
        co += step
    return out


def _build_nc(P0, P1):
    global EM_HEADS_IN_T1
    N0, N1 = P0 * 128, P1 * 128
    nc = bacc.Bacc(None)
    BF = mybir.dt.bfloat16
    F8 = mybir.dt.float8e4
    F32 = mybir.dt.float32
    I32 = mybir.dt.int32
    OP = mybir.AluOpType
    ACTF = mybir.ActivationFunctionType
    DR = mybir.MatmulPerfMode.DoubleRow
    KT = D // 128

    # inputs packed into per-phase blobs: one DMA descriptor each (the
    # Sync/Scalar queues recycle ~8 DMA semaphores; many small dma_starts
    # serialize on sem reuse and wreck the ramp)
    nb2a = KT * (N0 + W0)
    nb3a = KT * (N1 + H1)
    nb3b = (H1 // 128) * (W1 + N1) + KT * N0
    nb4 = KT * (512 + WH + 512)
    cst_d = nc.dram_tensor("cst", [128, 129], F32, kind="ExternalInput")
    b2a_d = nc.dram_tensor("b2a", [128, nb2a], F8, kind="ExternalInput")
    b3a_d = nc.dram_tensor("b3a", [128, nb3a], F8, kind="ExternalInput")
    b3b_d = nc.dram_tensor("b3b", [128, nb3b], F8, kind="ExternalInput")
    b4_d = nc.dram_tensor("b4", [128, nb4], F8, kind="ExternalInput")
    ra_d = nc.dram_tensor("ra", [128, 4 + P0 + 2 * P1], F32, kind="ExternalOutput")
    rv_d = nc.dram_tensor("rv", [128, 4 + P0 + 2 * P1], F32, kind="ExternalOutput")

    with tile.TileContext(nc) as tc:
        with (
            tc.tile_pool(name="const", bufs=1) as cp,
            tc.tile_pool(name="work", bufs=3) as wp,
            tc.tile_pool(name="psum", bufs=2, space="PSUM") as bigp,
        ):
            cst = cp.tile([128, 129], F32)
            b2a = cp.tile([128, nb2a], F8)
            b3a = cp.tile([128, nb3a], F8)
            b3b = cp.tile([128, nb3b], F8)
            b4 = cp.tile([128, nb4], F8)
            h1T8 = cp.tile([128, H1 // 128, N1], F8)
            ra = cp.tile([128, 4 + P0 + 2 * P1], F32)
            rv = cp.tile([128, 4 + P0 + 2 * P1], F32)

            def _cut(blob, off, n, k):
                ap = blob[:, off : off + n * k]
                return ap.rearrange("p (k n) -> p k n", k=k), off + n * k

            pidx = cst[:, 0:1]
            iota = cst[:, 1:129]
            o = 0
            inp0T, o = _cut(b2a, o, N0, KT)
            w0fT, o = _cut(b2a, o, W0, KT)
            o = 0
            inp1T, o = _cut(b3a, o, N1, KT)
            w1t1, o = _cut(b3a, o, H1, KT)
            o = 0
            w2t1, o = _cut(b3b, o, W1, H1 // 128)
            wgT1, o = _cut(b3b, o, N1, H1 // 128)
            wgT0f, o = _cut(b3b, o, N0, KT)
            o = 0
            inpH, o = _cut(b4, o, 512, KT)
            hwT, o = _cut(b4, o, WH, KT)
            wgH, o = _cut(b4, o, 512, KT)
            resha = ra[:, 0:4]
            res0a = ra[:, 4 : 4 + P0]
            res1a = ra[:, 4 + P0 : 4 + P0 + 2 * P1].rearrange(
                "p (m c) -> p m c", m=P1
            )
            reshv = rv[:, 0:4]
            res0v = rv[:, 4 : 4 + P0]
            res1v = rv[:, 4 + P0 : 4 + P0 + 2 * P1].rearrange(
                "p (m c) -> p m c", m=P1
            )

            # loads in first-use order; k-pair interleave for the hidden0
            # path so its kt-major GEMM can start on the first pair
            nc.sync.dma_start(cst[:], cst_d[:])
            nc.sync.dma_start(b2a[:], b2a_d[:])
            nc.sync.dma_start(b3a[:], b3a_d[:])
            nc.sync.dma_start(b3b[:], b3b_d[:])
            nc.sync.dma_start(b4[:], b4_d[:])

            # preload the exp table during the DMA ramp
            warm = wp.tile([128, 1], BF, tag="warm")
            nc.scalar.activation(warm[:], pidx, ACTF.Exp)

            # warm the PE HAM clock gate during the DMA ramp (fp32 matmuls
            # on the tiny iota tile); the dummy DVE read frees the slot
            psw = bigp.tile([128, 2048], F32, tag="big", name="ps")
            for _ in range(40):
                nc.tensor.matmul(psw[:, :128], iota, iota, start=True, stop=True)
            wsink = wp.tile([128, 1], F32, tag="wsink")
            nc.vector.tensor_scalar_mul(wsink[:], psw[:, 0:1], 0.0)

            def mm_block(ps, fchunks, nkt, lhsT_fn, rhs_fn):
                kts = list(range(0, nkt, 2))
                for co, cw in fchunks:
                    for ki, kt in enumerate(kts):
                        nc.tensor.matmul(
                            ps[:, co : co + cw],
                            lhsT_fn(kt),
                            rhs_fn(kt, co, cw),
                            start=(ki == 0),
                            stop=(ki == len(kts) - 1),
                            perf_mode=DR,
                        )

            def hid_job(inT, w1, hT8, mh, width):
                ps = bigp.tile([128, 2048], F32, tag="big", name="ps")
                mm_block(
                    ps, _chunks(width, 512), KT,
                    lambda kt: w1[:, kt : kt + 2, mh * 128 : (mh + 1) * 128],
                    lambda kt, co, cw: inT[:, kt : kt + 2, co : co + cw],
                )
                nc.vector.tensor_scalar_mul(
                    hT8[:, mh, :], ps[:, :width], HID_DESCALE * H_SCALE
                )

            def exp_job(ps, cw, scale, s_ap):
                sc_e = wp.tile([128, 2048], BF, tag="sc_e")
                nc.scalar.activation(
                    sc_e[:, :cw], ps[:, :cw], ACTF.Exp, scale=scale, accum_out=s_ap
                )

            def schraud_job(ps, cw, s_ap, folds=1):
                e32 = wp.tile([128, 2048], I32, tag="e32")
                nc.vector.tensor_scalar(
                    out=e32[:, :cw], in0=ps[:, :cw],
                    scalar1=SCH_K1, scalar2=SCH_B,
                    op0=OP.mult, op1=OP.add,
                )
                # fold the bitcast exps on gpsimd (1 or 2 folds; its per-op
                # launch overhead is ~2us), final accum on the DVE
                ef = e32[:].bitcast(F32)
                t9 = wp.tile([128, 960], F32, tag="t9")
                nc.gpsimd.tensor_tensor(
                    out=t9[:], in0=ef[:, 0:960], in1=ef[:, 960:1920], op=OP.add
                )
                fw = 960
                if folds == 2:
                    u4 = wp.tile([128, 480], F32, tag="u4")
                    nc.gpsimd.tensor_tensor(
                        out=u4[:], in0=t9[:, 0:480], in1=t9[:, 480:960], op=OP.add
                    )
                    t9, fw = u4, 480
                sc2 = wp.tile([128, 960], BF, tag="sc2")
                nc.vector.tensor_scalar(
                    out=sc2[:, :fw], in0=t9[:, :fw],
                    scalar1=1.0, scalar2=0.0, op0=OP.mult, op1=OP.add,
                    accum_out=s_ap,
                )

            def t0_job(m):
                ms = slice(m * 128, (m + 1) * 128)
                ps = bigp.tile([128, 2048], F32, tag="big", name="ps")
                mm_block(
                    ps, _chunks(W0, 512), KT,
                    lambda kt: inp0T[:, kt : kt + 2, ms],
                    lambda kt, co, cw: w0fT[:, kt : kt + 2, co : co + cw],
                )
                exp_job(ps, W0, DESCALE0, res0a[:, m : m + 1])

            def dg0_batch():
                ps = bigp.tile([128, 2048], F32, tag="big", name="ps")
                for m in range(P0):
                    ms = slice(m * 128, (m + 1) * 128)
                    for ki, kt in enumerate(range(0, KT, 2)):
                        nc.tensor.matmul(
                            ps[:, m * 128 : (m + 1) * 128],
                            inp0T[:, kt : kt + 2, ms],
                            wgT0f[:, kt : kt + 2, ms],
                            start=(ki == 0), stop=(kt + 2 >= KT),
                            perf_mode=DR,
                        )
                sd = exp_blk(ps[:, : P0 * 128], P0 * 128, DESCALE0)
                for m in range(P0):
                    extract(sd[:, m * 128 : (m + 1) * 128], res0v[:, m : m + 1])

            def exp_blk(ps_blk, cw, scale):
                # diag blocks: exp into SBUF (no accum) so the PSUM slot is
                # released by ACT alone; DVE extracts lag off-path.  The host
                # recovers the logit as sum of ln over cores (non-owner cores
                # contribute exp(0)=1).
                sc_d = wp.tile([128, max(P0, P1, 6) * 128], F32, tag="sc_d")
                nc.scalar.activation(sc_d[:, :cw], ps_blk, ACTF.Exp, scale=scale)
                return sc_d

            def extract(sb_blk, t_ap):
                sc_g = wp.tile([128, 128], BF, tag="sc_g")
                nc.vector.scalar_tensor_tensor(
                    out=sc_g[:],
                    in0=iota,
                    scalar=pidx,
                    in1=sb_blk,
                    op0=OP.is_equal,
                    op1=OP.mult,
                    accum_out=t_ap,
                )

            HW1 = W1 // 2  # 1920: A-slot logit cols; diag block at 1920:2048

            def t1A_job(m):
                ms = slice(m * 128, (m + 1) * 128)
                psA = bigp.tile([128, 2048], F32, tag="big", name="ps")
                mm_block(
                    psA, _chunks(HW1, 512), H1 // 128,
                    lambda kt: h1T8[:, kt : kt + 2, ms],
                    lambda kt, co, cw: w2t1[:, kt : kt + 2, co : co + cw],
                )
                exp_job(psA, HW1, DESCALE, res1a[:, m, 0:1])

            def t1B_job(m):
                ms = slice(m * 128, (m + 1) * 128)
                psB = bigp.tile([128, 2048], F32, tag="big", name="ps")
                mm_block(
                    psB, _chunks(HW1, 512), H1 // 128,
                    lambda kt: h1T8[:, kt : kt + 2, ms],
                    lambda kt, co, cw: w2t1[:, kt : kt + 2, HW1 + co : HW1 + co + cw],
                )
                if m < P1 - 1:
                    schraud_job(psB, HW1, res1v[:, m, 0:1])
                else:
                    exp_job(psB, HW1, DESCALE, res1a[:, m, 1:2])

            def dg1_batch():
                ps = bigp.tile([128, 2048], F32, tag="big", name="ps")
                for m in range(P1):
                    ms = slice(m * 128, (m + 1) * 128)
                    nc.tensor.matmul(
                        ps[:, m * 128 : (m + 1) * 128],
                        h1T8[:, 0:2, ms],
                        wgT1[:, 0:2, ms],
                        start=True, stop=True, perf_mode=DR,
                    )
                sd = exp_blk(ps[:, : P1 * 128], P1 * 128, DESCALE)
                for m in range(P1):
                    extract(sd[:, m * 128 : (m + 1) * 128], res1v[:, m, 1:2])

            def head_job(lt):
                ls = slice(lt * 128, (lt + 1) * 128)
                ps = bigp.tile([128, 2048], F32, tag="big", name="ps")
                mm_block(
                    ps, _chunks(WH, 512), KT,
                    lambda kt: inpH[:, kt : kt + 2, ls],
                    lambda kt, co, cw: hwT[:, kt : kt + 2, co : co + cw],
                )
                exp_job(ps, WH, HID_DESCALE, resha[:, lt : lt + 1])

            def dgh_batch():
                ps = bigp.tile([128, 2048], F32, tag="big", name="ps")
                for lt in range(4):
                    ls = slice(lt * 128, (lt + 1) * 128)
                    for ki, kt in enumerate(range(0, KT, 2)):
                        nc.tensor.matmul(
                            ps[:, lt * 128 : (lt + 1) * 128],
                            inpH[:, kt : kt + 2, ls],
                            wgH[:, kt : kt + 2, ls],
                            start=(ki == 0), stop=(kt + 2 >= KT),
                            perf_mode=DR,
                        )
                sd = exp_blk(ps[:, :512], 512, HID_DESCALE)
                for lt in range(4):
                    extract(sd[:, lt * 128 : (lt + 1) * 128], reshv[:, lt : lt + 1])

            # t0 first (its data lands first), hid1 after two t0 jobs,
            # then t1 iters; heads woven into late t1 iters (EM_HEADS_IN_T1)
            # or as a tail phase
            with nc.named_scope("main"):
                seq = [lambda m=m: t0_job(m) for m in range(min(2, P0))]
                seq += [
                    lambda: hid_job(inp1T, w1t1, h1T8, 0, N1),
                    lambda: hid_job(inp1T, w1t1, h1T8, 1, N1),
                ]
                seq += [lambda m=m: t0_job(m) for m in range(2, P0)]
                seq.append(dg0_batch)
                heads = [lambda lt=lt: head_job(lt) for lt in range(4)]
                for m in range(P1):
                    seq.append(lambda m=m: t1A_job(m))
                    seq.append(lambda m=m: t1B_job(m))
                    if m == 4:
                        seq.append(dg1_batch)
                    if EM_HEADS_IN_T1 and m >= P1 - 5 and heads:
                        seq.append(heads.pop(0))
                seq += heads
                seq.append(dgh_batch)
                for f in seq:
                    f()

            nc.sync.dma_start(ra_d[:], ra[:])
            nc.scalar.dma_start(rv_d[:], rv[:])

    nc.finalize()
    return nc


def _get_nc(P0, P1):
    key = (P0, P1, EM_HEADS_IN_T1)
    if key not in _CACHED_NC:
        _CACHED_NC[key] = _build_nc(P0, P1)
    return _CACHED_NC[key]


def _tiled(a2d):
    """[K, F] (K multiple of 128) -> contiguous [128, K//128, F]."""
    K, F = a2d.shape
    return np.ascontiguousarray(
        a2d.reshape(K // 128, 128, F).transpose(1, 0, 2)
    )


def _unpm(a):
    """[128, m] -> [m*128]."""
    return np.ascontiguousarray(a.T).reshape(-1)


def make_in_maps(inp, tgt, head_w, t0_w1, t0_w2, t1_w1, t1_w2):
    inp = np.asarray(inp, dtype=np.float32)
    tgt = np.asarray(tgt).astype(np.int64)

    in1 = (tgt >= C0) & (tgt < C1)
    in2 = tgt >= C1
    idx0 = np.where(in1)[0]
    idx1 = np.where(in2)[0]
    n0, n1 = len(idx0), len(idx1)
    P0 = max(1, -(-n0 // 128))
    P1 = max(1, -(-n1 // 128))
    idx0p = np.concatenate([idx0, np.zeros(P0 * 128 - n0, np.int64)])
    idx1p = np.concatenate([idx1, np.zeros(P1 * 128 - n1, np.int64)])

    inpT_s = (inp.T * IN_SCALE).astype(FP8)           # [D, N]
    inp0T = _tiled(np.ascontiguousarray(inpT_s[:, idx0p]))
    inp1T = _tiled(np.ascontiguousarray(inpT_s[:, idx1p]))
    w1t1 = _tiled((np.asarray(t1_w1, np.float32).T * W1_SCALE).astype(FP8))
    # fused tail0 projection: logits = inp @ (w2 @ w1).T directly (k=1024)
    w0f = (np.asarray(t0_w2, np.float32) @ np.asarray(t0_w1, np.float32))
    w0fT_full = (w0f.T * WF_SCALE).astype(FP8)        # [D, T0]

    hwT_full = np.zeros((D, HEAD_PAD), FP8)
    hwT_full[:, :HEAD] = (np.asarray(head_w, np.float32).T * W1_SCALE).astype(FP8)
    w2t1_full = np.zeros((H1, T1_PAD), FP8)
    w2t1_full[:, :T1] = (np.asarray(t1_w2, np.float32).T * W_SCALE).astype(FP8)

    gi = np.where(tgt < C0, tgt, np.where(tgt < C1, C0, C0 + 1))
    rel0 = tgt[idx0p] - C0
    rel1 = tgt[idx1p] - C1

    def _gathT(full, rel, own):
        # [K, osz] -> gathered [K, nrows], zeroed on non-owner cores
        g = np.ascontiguousarray(full[:, np.clip(rel, 0, full.shape[1] - 1)])
        g[:, ~own] = 0
        return _tiled(g)

    iota = np.broadcast_to(
        np.arange(128, dtype=np.float32)[None, :], (128, 128)
    ).copy()
    pidx = np.arange(128, dtype=np.float32)[:, None].copy()

    def _flat(*tiles):
        return np.ascontiguousarray(
            np.concatenate([t.reshape(128, -1) for t in tiles], axis=1)
        )

    cst = np.concatenate([pidx, iota], axis=1).astype(np.float32)
    b3a = _flat(inp1T, w1t1)
    in_maps = []
    for i in range(NCORES):
        j, h = i // 2, i % 2
        smp = slice(j * 512, (j + 1) * 512)
        gih = gi[smp]
        wgH_full = np.ascontiguousarray(hwT_full[:, gih])
        if h == 1:
            wgH_full = np.zeros_like(wgH_full)
        in_maps.append(
            {
                "cst": cst,
                "b2a": _flat(
                    inp0T, _tiled(w0fT_full[:, i * W0 : (i + 1) * W0])
                ),
                "b3a": b3a,
                "b3b": _flat(
                    _tiled(w2t1_full[:, i * W1 : (i + 1) * W1]),
                    _gathT(w2t1_full, rel1, (rel1 // W1) == i),
                    _gathT(w0fT_full, rel0, (rel0 // W0) == i),
                ),
                "b4": _flat(
                    _tiled(np.ascontiguousarray(inpT_s[:, smp])),
                    _tiled(hwT_full[:, h * WH : (h + 1) * WH]),
                    _tiled(wgH_full),
                ),
            }
        )
    return in_maps, tgt, (idx0, idx1, n0, n1, P0, P1)


def combine(results, tgt, meta):
    """per-core {'resh','res0','res1'} partials -> final [N] f32 NLL."""
    idx0, idx1, n0, n1, P0, P1 = meta
    Sh = np.zeros((128, MT), np.float64)
    Th = np.zeros((128, MT), np.float64)
    S0 = np.zeros((128, P0), np.float64)
    T0s = np.zeros((128, P0), np.float64)
    S1 = np.zeros((128, P1), np.float64)
    T1s = np.zeros((128, P1), np.float64)
    for i, r in enumerate(results):
        j = i // 2
        ra = np.asarray(r["ra"], np.float64)
        rv = np.asarray(r["rv"], np.float64)
        res1a = ra[:, 4 + P0 :].reshape(128, P1, 2)
        res1v = rv[:, 4 + P0 :].reshape(128, P1, 2)
        Sh[:, 4 * j : 4 * j + 4] += ra[:, 0:4]
        Th[:, 4 * j : 4 * j + 4] += np.log(rv[:, 0:4])
        S0 += ra[:, 4 : 4 + P0]
        T0s += np.log(rv[:, 4 : 4 + P0])
        S1 += res1a[:, :, 0] + res1a[:, :, 1] + res1v[:, :, 0]
        T1s += np.log(res1v[:, :, 1])

    # zero-padded cols: head pad on odd cores' halves (exp(0)=1 each);
    # tail1 pad all in core 7's Schraudolph B-chunk (approx exp(0)=SCH_E0)
    head_term = _unpm(Th) - np.log(_unpm(Sh) - PAD_H)
    lp0 = _unpm(T0s) - np.log(_unpm(S0))
    padc = np.where(np.arange(P1) < P1 - 1, PAD_1 * SCH_E0, float(PAD_1))
    lp1 = _unpm(T1s) - np.log(_unpm(S1 - padc[None, :]))

    out = head_term
    out[idx0] += lp0[:n0]
    out[idx1] += lp1[:n1]
    return (-out).astype(np.float32)


def kernel(inp, tgt, head_w, t0_w1, t0_w2, t1_w1, t1_w2):
    global LAST_RESULT
    in_maps, tgt64, meta = make_in_maps(
        inp, tgt, head_w, t0_w1, t0_w2, t1_w1, t1_w2
    )
    nc = _get_nc(meta[4], meta[5])
    out = run_bass_kernel_spmd(
        nc, in_maps, core_ids=list(range(NCORES)), trace=TRACE
    )
    LAST_RESULT = out
    return combine(out.results, tgt64, meta)
